# revision 12
# baseline (speedup 1.0000x reference)
"""MinGRU layer (Heinsen-scan) Trainium2 kernel.

Reference computation (per batch b):
    hg = x @ W_hg                    # [T, 2*Di] -> hidden | gate
    z_t = sigmoid(gate_t)
    g(x) = where(x>=0, x+0.5, sigmoid(x)) == max(x+0.5, sigmoid(x))  (exact)
    h_t = (1-z_t) * h_{t-1} + z_t * g(hidden_t)      (positive affine scan)
    out = h @ W_out ;  next_hidden = h_T

The reference does the scan in log space for stability; since every term is
positive the plain linear-space affine recurrence is numerically equivalent
in fp32 (validated ~1e-4 max rel err vs the log-space reference).

Sharding over 8 cores: (batch b in 0..3) x (dim_inner half j in 0..1).
Each core:
  * transposes its x[b] slab on the PE (scan needs [channel, token] layout),
  * computes hidden/gate for its 1024 channels (f32r matmuls, full PE rate),
  * a_t = sigmoid(-gate), b_t = sigmoid(gate)*max(hidden+0.5, sigmoid(hidden)),
  * h = affine scan via tensor_tensor_scan (per-channel recurrence along T),
  * partial out projection in [D, token] layout (h tiles feed the second
    matmul directly as the moving operand),
  * pairwise ReduceScatter (cores 2b, 2b+1) sums the two half-dim_inner
    partial projections; each core keeps half the D rows.
Host reassembles the [D, T] transposed shards into [B, T, D].
"""

import numpy as np

import concourse.bass as bass
import concourse.bacc as bacc
import concourse.mybir as mybir
import concourse.tile as tile
from concourse.masks import make_identity

F32 = mybir.dt.float32
F32R = mybir.dt.float32r
AF = mybir.ActivationFunctionType
OP = mybir.AluOpType

B, T, D = 4, 4096, 1024
DI = 2048            # dim_inner
DJ = DI // 2         # per-core half of dim_inner
NTH = 4              # token phases
THL = T // NTH       # tokens per phase (1024)
NK = D // 128        # contraction tiles for x @ W_hg (8)
NC_ = DJ // 128      # channel tiles per core (8)
ND = D // 128        # output-D tiles (8)

_CACHE = {}


def _r(ap):
    # float32r = fp32 bits on the fast (1 cycle/row) PE streaming path
    return ap.bitcast(F32R)


def _build(use_collective=True):
    nc = bacc.Bacc(None)
    x_d = nc.declare_dram_parameter("x", [T, D], F32, isOutput=False)
    whg_d = nc.declare_dram_parameter("whg", [D, 2 * DJ], F32R, isOutput=False)
    wout_d = nc.declare_dram_parameter("wout", [DJ, D], F32R, isOutput=False)
    ph_d = nc.declare_dram_parameter("ph", [DJ], F32, isOutput=False)
    # 4 phase-blocks of [512 D-rows, THL tokens] (this core's RS share)
    if use_collective:
        part_d = nc.declare_dram_parameter("part", [NTH * 512, THL], F32, isOutput=True)
    else:
        part_d = nc.declare_dram_parameter("part", [NTH * D, THL], F32, isOutput=True)
    nh_d = nc.declare_dram_parameter("nh", [DJ], F32, isOutput=True)

    groups = [[0, 1], [2, 3], [4, 5], [6, 7]]

    with tile.TileContext(nc) as tc:
        with (
            tc.tile_pool(name="const", bufs=1) as constp,
            tc.tile_pool(name="whg", bufs=1) as whgp,
            tc.tile_pool(name="xt", bufs=1) as xtp,
            tc.tile_pool(name="h", bufs=1) as hp,
            tc.tile_pool(name="stage", bufs=4) as stp,
            tc.tile_pool(name="wo", bufs=2) as wop,
            tc.tile_pool(name="ew", bufs=2) as ewp,
            tc.tile_pool(name="oe", bufs=2) as oep,
            tc.tile_pool(name="ps", bufs=4, space="PSUM") as psp,
            tc.tile_pool(name="dram", bufs=2, space="DRAM") as drp,
        ):
            ident = constp.tile([128, 128], F32, tag="ident")
            make_identity(nc, ident[:])
            phc = constp.tile([128, NC_], F32, tag="phc")
            nc.sync.dma_start(phc[:], ph_d[:].rearrange("(c p) -> p c", p=128))
            carry = constp.tile([128, NC_], F32, tag="carry")


            whg_sb = whgp.tile([128, NK, 2 * DJ], F32R, tag="whg")
            nc.sync.dma_start(
                whg_sb[:], whg_d[:].rearrange("(k p) m -> p k m", p=128)
            )

            for th in range(NTH):
                t0 = th * THL

                # ---- transpose x[t0:t0+THL, :] into [D, tok] tiles ----
                xt = [xtp.tile([128, THL], F32R, tag=f"xt{k}", name=f"xt{k}_{th}") for k in range(NK)]
                for tp in range(THL // 256):
                    s0 = stp.tile([128, D], F32, tag="stage")
                    s1 = stp.tile([128, D], F32, tag="stage")
                    r0 = t0 + tp * 256
                    nc.sync.dma_start(s0[:], x_d[r0 : r0 + 128, :])
                    nc.sync.dma_start(s1[:], x_d[r0 + 128 : r0 + 256, :])
                    for k in range(NK):
                        pt = psp.tile([128, 1024], F32, tag="ps")
                        nc.tensor.transpose(
                            pt[:, 0:128], s0[:, k * 128 : (k + 1) * 128], ident[:]
                        )
                        nc.tensor.transpose(
                            pt[:, 128:256], s1[:, k * 128 : (k + 1) * 128], ident[:]
                        )
                        nc.scalar.copy(
                            xt[k][:, tp * 256 : (tp + 1) * 256], pt[:, 0:256]
                        )

                # ---- hidden/gate matmuls + elementwise + scan ----
                ht = [hp.tile([128, THL], F32R, tag=f"h{c}", name=f"h{c}_{th}") for c in range(NC_)]
                for c in range(NC_):
                    pht = psp.tile([128, 1024], F32, tag="ps")
                    pgt = psp.tile([128, 1024], F32, tag="ps")
                    for k in range(NK):
                        w1 = whg_sb[:, k, c * 128 : (c + 1) * 128]
                        w2 = whg_sb[:, k, DJ + c * 128 : DJ + (c + 1) * 128]
                        for q in range(2):
                            sl = slice(q * 512, (q + 1) * 512)
                            nc.tensor.matmul(
                                pht[:, sl], w1, xt[k][:, sl],
                                start=(k == 0), stop=(k == NK - 1),
                            )
                            nc.tensor.matmul(
                                pgt[:, sl], w2, xt[k][:, sl],
                                start=(k == 0), stop=(k == NK - 1),
                            )
                    for q in range(2):
                        sl = slice(q * 512, (q + 1) * 512)
                        sg = ewp.tile([128, 512], F32, tag="sg")
                        sh = ewp.tile([128, 512], F32, tag="sh")
                        at = ewp.tile([128, 512], F32, tag="a")
                        gh = ewp.tile([128, 512], F32, tag="gh")
                        bt = ewp.tile([128, 512], F32, tag="b")
                        nc.scalar.activation(sg[:], pgt[:, sl], AF.Sigmoid)
                        nc.scalar.activation(at[:], pgt[:, sl], AF.Sigmoid, scale=-1.0)
                        nc.scalar.activation(sh[:], pht[:, sl], AF.Sigmoid)
                        # g(hidden) = max(hidden + 0.5, sigmoid(hidden))
                        nc.vector.scalar_tensor_tensor(
                            gh[:], pht[:, sl], 0.5, sh[:], OP.add, OP.max
                        )
                        nc.vector.tensor_mul(bt[:], sg[:], gh[:])
                        if th == 0 and q == 0:
                            init = phc[:, c : c + 1]
                        elif q == 0:
                            init = carry[:, c : c + 1]
                        else:
                            init = ht[c][:, 511:512]
                        # state = a*state + b along the token axis
                        nc.vector.tensor_tensor_scan(
                            ht[c][:, sl], at[:], bt[:], init, OP.mult, OP.add
                        )
                    nc.vector.tensor_copy(carry[:, c : c + 1], ht[c][:, THL - 1 : THL])

                # ---- partial out projection, [D, tok] layout ----
                pd = drp.tile([D, THL], F32, tag="pd")
                for d in range(ND):
                    wo = wop.tile([128, NK, 128], F32R, tag="wo")
                    nc.sync.dma_start(
                        wo[:],
                        wout_d[:, d * 128 : (d + 1) * 128].rearrange(
                            "(k p) m -> p k m", p=128
                        ),
                    )
                    po = psp.tile([128, 1024], F32, tag="ps")
                    for k in range(NK):
                        for q in range(2):
                            sl = slice(q * 512, (q + 1) * 512)
                            nc.tensor.matmul(
                                po[:, sl], wo[:, k, :], ht[k][:, sl],
                                start=(k == 0), stop=(k == NK - 1),
                            )
                    ov = oep.tile([128, THL], F32, tag="oe")
                    nc.vector.tensor_copy(ov[:], po[:])
                    nc.sync.dma_start(pd[d * 128 : (d + 1) * 128, :], ov[:])

                if use_collective:
                    rs = drp.tile([512, THL], F32, tag="rs")
                    nc.gpsimd.collective_compute(
                        "ReduceScatter",
                        OP.add,
                        replica_groups=groups,
                        ins=[pd[:]],
                        outs=[rs[:]],
                    )
                    nc.sync.dma_start(part_d[th * 512 : (th + 1) * 512, :], rs[:])
                else:
                    nc.sync.dma_start(part_d[th * D : (th + 1) * D, :], pd[:])

            nc.sync.dma_start(nh_d[:].rearrange("(c p) -> p c", p=128), carry[:])

    nc.compile()
    return nc


def get_nc(use_collective=True):
    key = ("nc", use_collective)
    if key not in _CACHE:
        _CACHE[key] = _build(use_collective)
    return _CACHE[key]


def make_in_maps(x, prev_hidden, W_hg, W_out):
    x = np.ascontiguousarray(x, dtype=np.float32)
    prev_hidden = np.ascontiguousarray(prev_hidden, dtype=np.float32)
    W_hg = np.ascontiguousarray(W_hg, dtype=np.float32)
    W_out = np.ascontiguousarray(W_out, dtype=np.float32)
    in_maps = []
    for cid in range(8):
        b, j = cid // 2, cid % 2
        whg_j = np.concatenate(
            [W_hg[:, j * DJ : (j + 1) * DJ], W_hg[:, DI + j * DJ : DI + (j + 1) * DJ]],
            axis=1,
        )
        in_maps.append(
            {
                "x": np.ascontiguousarray(x[b]),
                "whg": np.ascontiguousarray(whg_j),
                "wout": np.ascontiguousarray(W_out[j * DJ : (j + 1) * DJ, :]),
                "ph": np.ascontiguousarray(prev_hidden[b, j * DJ : (j + 1) * DJ]),
            }
        )
    return in_maps


def assemble(results):
    """results: list of 8 dicts with 'part' [NTH*512, THL] and 'nh' [DJ]."""
    out = np.empty((B, T, D), dtype=np.float32)
    nh = np.empty((B, DI), dtype=np.float32)
    for b in range(B):
        p0 = np.asarray(results[2 * b]["part"]).reshape(NTH * 512, THL)
        p1 = np.asarray(results[2 * b + 1]["part"]).reshape(NTH * 512, THL)
        outT = np.empty((D, T), dtype=np.float32)
        for th in range(NTH):
            outT[0:512, th * THL : (th + 1) * THL] = p0[th * 512 : (th + 1) * 512]
            outT[512:1024, th * THL : (th + 1) * THL] = p1[th * 512 : (th + 1) * 512]
        out[b] = outT.T
        nh[b, :DJ] = np.asarray(results[2 * b]["nh"]).reshape(DJ)
        nh[b, DJ:] = np.asarray(results[2 * b + 1]["nh"]).reshape(DJ)
    return out, nh


def run(x, prev_hidden, W_hg, W_out, trace=False, use_collective=True, **spmd_kwargs):
    from concourse.bass_utils import run_bass_kernel_spmd

    nc = get_nc(use_collective)
    in_maps = make_in_maps(x, prev_hidden, W_hg, W_out)
    br = run_bass_kernel_spmd(
        nc, in_maps, list(range(8)), trace=trace, **spmd_kwargs
    )
    if use_collective:
        out, nh = assemble(br.results)
    else:
        out, nh = assemble_nocc(br.results)
    return (out, nh), br


def assemble_nocc(results):
    """Debug path: 'part' holds the full unreduced [D, THL] per phase."""
    out = np.empty((B, T, D), dtype=np.float32)
    nh = np.empty((B, DI), dtype=np.float32)
    for b in range(B):
        p0 = np.asarray(results[2 * b]["part"]).reshape(NTH, D, THL)
        p1 = np.asarray(results[2 * b + 1]["part"]).reshape(NTH, D, THL)
        ps = p0 + p1
        outT = np.concatenate([ps[th] for th in range(NTH)], axis=1)
        out[b] = outT.T
        nh[b, :DJ] = np.asarray(results[2 * b]["nh"]).reshape(DJ)
        nh[b, DJ:] = np.asarray(results[2 * b + 1]["nh"]).reshape(DJ)
    return out, nh


def kernel(x, prev_hidden, W_hg, W_out):
    (out, nh), _ = run(x, prev_hidden, W_hg, W_out, trace=False)
    return (out, nh)


# revision 13
# speedup vs baseline: 1.0754x; 1.0754x over previous
"""MinGRU layer (Heinsen-scan) Trainium2 kernel.

Reference computation (per batch b):
    hg = x @ W_hg                    # [T, 2*Di] -> hidden | gate
    z_t = sigmoid(gate_t)
    g(x) = where(x>=0, x+0.5, sigmoid(x)) == max(x+0.5, sigmoid(x))  (exact)
    h_t = (1-z_t) * h_{t-1} + z_t * g(hidden_t)      (positive affine scan)
    out = h @ W_out ;  next_hidden = h_T

The reference does the scan in log space for stability; since every term is
positive the plain linear-space affine recurrence is numerically equivalent
in fp32 (validated ~1e-4 max rel err vs the log-space reference).

Sharding over 8 cores: (batch b in 0..3) x (dim_inner half j in 0..1).
Each core:
  * transposes its x[b] slab on the PE (scan needs [channel, token] layout),
  * computes hidden/gate for its 1024 channels (f32r matmuls, full PE rate),
  * a_t = sigmoid(-gate), b_t = sigmoid(gate)*max(hidden+0.5, sigmoid(hidden)),
  * h = affine scan via tensor_tensor_scan (per-channel recurrence along T),
  * partial out projection in [D, token] layout (h tiles feed the second
    matmul directly as the moving operand),
  * pairwise ReduceScatter (cores 2b, 2b+1) sums the two half-dim_inner
    partial projections; each core keeps half the D rows.
Host reassembles the [D, T] transposed shards into [B, T, D].
"""

import numpy as np

import concourse.bass as bass
import concourse.bacc as bacc
import concourse.mybir as mybir
import concourse.tile as tile
from concourse.masks import make_identity

F32 = mybir.dt.float32
F32R = mybir.dt.float32r
AF = mybir.ActivationFunctionType
OP = mybir.AluOpType

B, T, D = 4, 4096, 1024
DI = 2048            # dim_inner
DJ = DI // 2         # per-core half of dim_inner
NTH = 4              # token phases
THL = T // NTH       # tokens per phase (1024)
NK = D // 128        # contraction tiles for x @ W_hg (8)
NC_ = DJ // 128      # channel tiles per core (8)
ND = D // 128        # output-D tiles (8)

_CACHE = {}


def _r(ap):
    # float32r = fp32 bits on the fast (1 cycle/row) PE streaming path
    return ap.bitcast(F32R)


def _build(use_collective=True):
    nc = bacc.Bacc(None)
    x_d = nc.declare_dram_parameter("x", [T, D], F32, isOutput=False)
    whg_d = nc.declare_dram_parameter("whg", [D, 2 * DJ], F32R, isOutput=False)
    wout_d = nc.declare_dram_parameter("wout", [DJ, D], F32R, isOutput=False)
    ph_d = nc.declare_dram_parameter("ph", [DJ], F32, isOutput=False)
    # 4 phase-blocks of [512 D-rows, THL tokens] (this core's RS share)
    if use_collective:
        part_d = nc.declare_dram_parameter("part", [NTH * 512, THL], F32, isOutput=True)
    else:
        part_d = nc.declare_dram_parameter("part", [NTH * D, THL], F32, isOutput=True)
    nh_d = nc.declare_dram_parameter("nh", [DJ], F32, isOutput=True)

    groups = [[0, 1], [2, 3], [4, 5], [6, 7]]

    with tile.TileContext(nc) as tc:
        with (
            tc.tile_pool(name="const", bufs=1) as constp,
            tc.tile_pool(name="whg", bufs=1) as whgp,
            tc.tile_pool(name="xt", bufs=1) as xtp,
            tc.tile_pool(name="h", bufs=1) as hp,
            tc.tile_pool(name="stage", bufs=6) as stp,
            tc.tile_pool(name="wo", bufs=2) as wop,
            tc.tile_pool(name="ew", bufs=2) as ewp,
            tc.tile_pool(name="oe", bufs=2) as oep,
            tc.tile_pool(name="ps", bufs=4, space="PSUM") as psp,
            tc.tile_pool(name="dram", bufs=2, space="DRAM") as drp,
        ):
            ident = constp.tile([128, 128], F32, tag="ident")
            make_identity(nc, ident[:])
            phc = constp.tile([128, NC_], F32, tag="phc")
            nc.sync.dma_start(phc[:], ph_d[:].rearrange("(c p) -> p c", p=128))
            carry = constp.tile([128, NC_], F32, tag="carry")


            whg_sb = whgp.tile([128, NK, 2 * DJ], F32R, tag="whg")

            def emit_transposes(th):
                # transpose x[th*THL:(th+1)*THL, :] into [D, tok] tiles
                t0 = th * THL
                xt = [
                    xtp.tile([128, THL], F32R, tag=f"xt{k}", name=f"xt{k}_{th}")
                    for k in range(NK)
                ]
                for tp in range(THL // 256):
                    s0 = stp.tile([128, D], F32, tag="stage", name=f"s0_{th}_{tp}")
                    s1 = stp.tile([128, D], F32, tag="stage", name=f"s1_{th}_{tp}")
                    r0 = t0 + tp * 256
                    nc.sync.dma_start(s0[:], x_d[r0 : r0 + 128, :])
                    nc.sync.dma_start(s1[:], x_d[r0 + 128 : r0 + 256, :])
                    if th == 0 and tp == 0:
                        # W_hg load goes after the first stage loads so the
                        # opening transposes are not starved behind 16MB
                        for k in range(NK):
                            nc.sync.dma_start(
                                whg_sb[:, k, :],
                                whg_d[k * 128 : (k + 1) * 128, :],
                            )
                    for k in range(NK):
                        pt = psp.tile([128, 1024], F32, tag="ps", name=f"pt_{th}_{tp}_{k}")
                        nc.tensor.transpose(
                            pt[:, 0:128], s0[:, k * 128 : (k + 1) * 128], ident[:]
                        )
                        nc.tensor.transpose(
                            pt[:, 128:256], s1[:, k * 128 : (k + 1) * 128], ident[:]
                        )
                        nc.scalar.copy(
                            xt[k][:, tp * 256 : (tp + 1) * 256], pt[:, 0:256]
                        )
                return xt

            xt = emit_transposes(0)
            for th in range(NTH):
                t0 = th * THL

                # ---- hidden/gate matmuls + elementwise + scan ----
                ht = [hp.tile([128, THL], F32R, tag=f"h{c}", name=f"h{c}_{th}") for c in range(NC_)]
                for c in range(NC_):
                    pht = psp.tile([128, 1024], F32, tag="ps")
                    pgt = psp.tile([128, 1024], F32, tag="ps")
                    for k in range(NK):
                        w1 = whg_sb[:, k, c * 128 : (c + 1) * 128]
                        w2 = whg_sb[:, k, DJ + c * 128 : DJ + (c + 1) * 128]
                        for q in range(2):
                            sl = slice(q * 512, (q + 1) * 512)
                            nc.tensor.matmul(
                                pht[:, sl], w1, xt[k][:, sl],
                                start=(k == 0), stop=(k == NK - 1),
                            )
                        for q in range(2):
                            sl = slice(q * 512, (q + 1) * 512)
                            nc.tensor.matmul(
                                pgt[:, sl], w2, xt[k][:, sl],
                                start=(k == 0), stop=(k == NK - 1),
                            )
                    for q in range(2):
                        sl = slice(q * 512, (q + 1) * 512)
                        sg = ewp.tile([128, 512], F32, tag="sg")
                        sh = ewp.tile([128, 512], F32, tag="sh")
                        at = ewp.tile([128, 512], F32, tag="a")
                        gh = ewp.tile([128, 512], F32, tag="gh")
                        bt = ewp.tile([128, 512], F32, tag="b")
                        nc.scalar.activation(sg[:], pgt[:, sl], AF.Sigmoid)
                        nc.scalar.activation(at[:], pgt[:, sl], AF.Sigmoid, scale=-1.0)
                        nc.scalar.activation(sh[:], pht[:, sl], AF.Sigmoid)
                        # g(hidden) = max(hidden + 0.5, sigmoid(hidden))
                        nc.vector.scalar_tensor_tensor(
                            gh[:], pht[:, sl], 0.5, sh[:], OP.add, OP.max
                        )
                        nc.vector.tensor_mul(bt[:], sg[:], gh[:])
                        if th == 0 and q == 0:
                            init = phc[:, c : c + 1]
                        elif q == 0:
                            init = carry[:, c : c + 1]
                        else:
                            init = ht[c][:, 511:512]
                        # state = a*state + b along the token axis
                        nc.vector.tensor_tensor_scan(
                            ht[c][:, sl], at[:], bt[:], init, OP.mult, OP.add
                        )
                    nc.vector.tensor_copy(carry[:, c : c + 1], ht[c][:, THL - 1 : THL])

                xt_next = emit_transposes(th + 1) if th + 1 < NTH else None

                # ---- partial out projection, [D, tok] layout ----
                pd = drp.tile([D, THL], F32, tag="pd")
                for d in range(ND):
                    wo = wop.tile([128, NK, 128], F32R, tag="wo")
                    nc.sync.dma_start(
                        wo[:],
                        wout_d[:, d * 128 : (d + 1) * 128].rearrange(
                            "(k p) m -> p k m", p=128
                        ),
                    )
                    po = psp.tile([128, 1024], F32, tag="ps")
                    for k in range(NK):
                        for q in range(2):
                            sl = slice(q * 512, (q + 1) * 512)
                            nc.tensor.matmul(
                                po[:, sl], wo[:, k, :], ht[k][:, sl],
                                start=(k == 0), stop=(k == NK - 1),
                            )
                    ov = oep.tile([128, THL], F32, tag="oe")
                    nc.vector.tensor_copy(ov[:], po[:])
                    nc.gpsimd.dma_start(pd[d * 128 : (d + 1) * 128, :], ov[:])

                if use_collective:
                    rs = drp.tile([512, THL], F32, tag="rs")
                    nc.gpsimd.collective_compute(
                        "ReduceScatter",
                        OP.add,
                        replica_groups=groups,
                        ins=[pd[:]],
                        outs=[rs[:]],
                    )
                    nc.gpsimd.dma_start(part_d[th * 512 : (th + 1) * 512, :], rs[:])
                else:
                    nc.sync.dma_start(part_d[th * D : (th + 1) * D, :], pd[:])
                if xt_next is not None:
                    xt = xt_next

            nc.gpsimd.dma_start(nh_d[:].rearrange("(c p) -> p c", p=128), carry[:])

    nc.compile()
    return nc


def get_nc(use_collective=True):
    key = ("nc", use_collective)
    if key not in _CACHE:
        _CACHE[key] = _build(use_collective)
    return _CACHE[key]


def make_in_maps(x, prev_hidden, W_hg, W_out):
    x = np.ascontiguousarray(x, dtype=np.float32)
    prev_hidden = np.ascontiguousarray(prev_hidden, dtype=np.float32)
    W_hg = np.ascontiguousarray(W_hg, dtype=np.float32)
    W_out = np.ascontiguousarray(W_out, dtype=np.float32)
    in_maps = []
    for cid in range(8):
        b, j = cid // 2, cid % 2
        whg_j = np.concatenate(
            [W_hg[:, j * DJ : (j + 1) * DJ], W_hg[:, DI + j * DJ : DI + (j + 1) * DJ]],
            axis=1,
        )
        in_maps.append(
            {
                "x": np.ascontiguousarray(x[b]),
                "whg": np.ascontiguousarray(whg_j),
                "wout": np.ascontiguousarray(W_out[j * DJ : (j + 1) * DJ, :]),
                "ph": np.ascontiguousarray(prev_hidden[b, j * DJ : (j + 1) * DJ]),
            }
        )
    return in_maps


def assemble(results):
    """results: list of 8 dicts with 'part' [NTH*512, THL] and 'nh' [DJ]."""
    out = np.empty((B, T, D), dtype=np.float32)
    nh = np.empty((B, DI), dtype=np.float32)
    for b in range(B):
        p0 = np.asarray(results[2 * b]["part"]).reshape(NTH * 512, THL)
        p1 = np.asarray(results[2 * b + 1]["part"]).reshape(NTH * 512, THL)
        outT = np.empty((D, T), dtype=np.float32)
        for th in range(NTH):
            outT[0:512, th * THL : (th + 1) * THL] = p0[th * 512 : (th + 1) * 512]
            outT[512:1024, th * THL : (th + 1) * THL] = p1[th * 512 : (th + 1) * 512]
        out[b] = outT.T
        nh[b, :DJ] = np.asarray(results[2 * b]["nh"]).reshape(DJ)
        nh[b, DJ:] = np.asarray(results[2 * b + 1]["nh"]).reshape(DJ)
    return out, nh


def run(x, prev_hidden, W_hg, W_out, trace=False, use_collective=True, **spmd_kwargs):
    from concourse.bass_utils import run_bass_kernel_spmd

    nc = get_nc(use_collective)
    in_maps = make_in_maps(x, prev_hidden, W_hg, W_out)
    br = run_bass_kernel_spmd(
        nc, in_maps, list(range(8)), trace=trace, **spmd_kwargs
    )
    if use_collective:
        out, nh = assemble(br.results)
    else:
        out, nh = assemble_nocc(br.results)
    return (out, nh), br


def assemble_nocc(results):
    """Debug path: 'part' holds the full unreduced [D, THL] per phase."""
    out = np.empty((B, T, D), dtype=np.float32)
    nh = np.empty((B, DI), dtype=np.float32)
    for b in range(B):
        p0 = np.asarray(results[2 * b]["part"]).reshape(NTH, D, THL)
        p1 = np.asarray(results[2 * b + 1]["part"]).reshape(NTH, D, THL)
        ps = p0 + p1
        outT = np.concatenate([ps[th] for th in range(NTH)], axis=1)
        out[b] = outT.T
        nh[b, :DJ] = np.asarray(results[2 * b]["nh"]).reshape(DJ)
        nh[b, DJ:] = np.asarray(results[2 * b + 1]["nh"]).reshape(DJ)
    return out, nh


def kernel(x, prev_hidden, W_hg, W_out):
    (out, nh), _ = run(x, prev_hidden, W_hg, W_out, trace=False)
    return (out, nh)
